# revision 1
# baseline (speedup 1.0000x reference)
"""Trainium2 Bass kernel for nn_ASTDecoder (banded-adjacency GCN stack).

Structure exploited (graph is fixed: nodes i<->i+1, i<->i+2 within each
256-node graph, plus self loops; initial node features are identical for
all nodes of a graph):
  * deg = [3,4,5,...,5,4,3]; interior rows of the normalized adjacency
    sum to exactly 1, so nodes far from the graph boundary keep a single
    per-graph value through all 3 conv layers.  After 3 layers only nodes
    0..7 and 248..255 differ from the per-graph constant; the two ends
    are mirror images of each other.
  * Per core (256 graphs): compute the per-graph constant chain and one
    16-node boundary strip exactly, then write the output: 240 interior
    rows per graph are a broadcast of one 512B row (the bulk of the 32MB
    per-core output traffic), 16 edge rows come from the strip.
"""

import os
import sys

import numpy as np

for _p in ("/opt/trn_rl_repo", "/root/.axon_site/_ro/trn_rl_repo"):
    if os.path.isdir(_p) and _p not in sys.path:
        sys.path.insert(0, _p)

from concourse import bacc, bass, mybir, tile
from concourse.bass import broadcast_tensor_aps
from concourse.bass_utils import run_bass_kernel_spmd

B, E, H, O, N = 2048, 256, 64, 128, 256
CORES = 8
G = B // CORES  # graphs per core
S = 16          # boundary strip width
AF = mybir.ActivationFunctionType
ALU = mybir.AluOpType
DT = mybir.dt.float32

USE_BCAST = True  # interior write: broadcast-read DMA vs replicate-in-SBUF


def _consts():
    deg = np.full(N, 5.0, np.float32)
    deg[[0, -1]] = 3.0
    deg[[1, -2]] = 4.0
    dinv = (1.0 / np.sqrt(deg)).astype(np.float32)
    q = np.float32(dinv[128] * dinv[128])
    c_int = np.float32(np.float32(5.0) * q)
    Bm = np.zeros((S, S), np.float32)
    for i in range(S):
        for j in range(S):
            if abs(i - j) <= 2:
                Bm[i, j] = np.float32(dinv[i] * dinv[j])
    return q, c_int, Bm


_Q, _CINT, _BM = _consts()
_NC = None


def _build():
    nc = bacc.Bacc("TRN2", target_bir_lowering=False, debug=False, num_devices=CORES)
    emb = nc.dram_tensor("emb", [G, E], DT, kind="ExternalInput")
    w_emb = nc.dram_tensor("w_emb", [E, H], DT, kind="ExternalInput")
    conv_w = nc.dram_tensor("conv_w", [3, H, H], DT, kind="ExternalInput")
    w_out = nc.dram_tensor("w_out", [H, O], DT, kind="ExternalInput")
    bias_h = nc.dram_tensor("bias_h", [H, 4], DT, kind="ExternalInput")
    b_out_t = nc.dram_tensor("b_out_t", [O, 1], DT, kind="ExternalInput")
    ident = nc.dram_tensor("ident", [128, 128], DT, kind="ExternalInput")
    out = nc.dram_tensor("out", [G, N, O], DT, kind="ExternalOutput")

    with tile.TileContext(nc) as tc:
        with (
            tc.tile_pool(name="const", bufs=1) as constp,
            tc.tile_pool(name="embp", bufs=1) as embp,
            tc.tile_pool(name="stripp", bufs=2) as stripp,
            tc.tile_pool(name="hsp", bufs=1) as hsp,
            tc.tile_pool(name="scr", bufs=1) as scr,
            tc.tile_pool(name="chain", bufs=2) as chainp,
            tc.tile_pool(name="outp", bufs=1) as outp,
            tc.tile_pool(name="etp", bufs=4) as etp,
            tc.tile_pool(name="ps", bufs=2, space="PSUM") as ps,
        ):
            # ---- constants into SBUF
            wemb_sb = constp.tile([128, 2, H], DT)
            nc.sync.dma_start(wemb_sb[:], w_emb.ap().rearrange("(c p) h -> p c h", p=128))
            wl_sb = constp.tile([H, 3, H], DT)
            nc.sync.dma_start(wl_sb[:], conv_w.ap().rearrange("l h k -> h l k"))
            wout_sb = constp.tile([H, O], DT)
            nc.sync.dma_start(wout_sb[:], w_out.ap())
            biash_sb = constp.tile([H, 4], DT)
            nc.sync.dma_start(biash_sb[:], bias_h.ap())
            bout_sb = constp.tile([O, 1], DT)
            nc.sync.dma_start(bout_sb[:], b_out_t.ap())
            ident_sb = constp.tile([128, 128], DT)
            nc.sync.dma_start(ident_sb[:], ident.ap())

            # ---- embedding load + PE transpose to [e, g]
            emb_sb = embp.tile([128, 2, E], DT)
            nc.sync.dma_start(emb_sb[:], emb.ap().rearrange("(c p) e -> p c e", p=128))
            embT = embp.tile([128, 2, G], DT)
            for ec in range(2):
                for gc in range(2):
                    tp = ps.tile([128, 128], DT, tag="tr")
                    nc.tensor.transpose(
                        tp[:], emb_sb[:, gc, ec * 128:(ec + 1) * 128], ident_sb[:]
                    )
                    nc.vector.tensor_copy(embT[:, ec, gc * 128:(gc + 1) * 128], tp[:])

            # ---- init = W_emb.T @ emb.T + b_emb   (layout [h, g])
            ip = ps.tile([H, G], DT, tag="mm")
            nc.tensor.matmul(ip[:], wemb_sb[:, 0, :], embT[:, 0, :], start=True, stop=False)
            nc.tensor.matmul(ip[:], wemb_sb[:, 1, :], embT[:, 1, :], start=False, stop=True)
            v = chainp.tile([H, G], DT, tag="v")
            nc.scalar.activation(v[:], ip[:], AF.Identity, bias=biash_sb[:, 0:1], scale=1.0)

            # strip[h, i, g] := v for all 16 strip rows
            strip = stripp.tile([H, S, G], DT, tag="strip")
            nc.gpsimd.tensor_copy(strip[:, 0, :], v[:])
            nc.gpsimd.tensor_copy(strip[:, 1, :], strip[:, 0, :])
            nc.gpsimd.tensor_copy(strip[:, 2:4, :], strip[:, 0:2, :])
            nc.gpsimd.tensor_copy(strip[:, 4:8, :], strip[:, 0:4, :])
            nc.gpsimd.tensor_copy(strip[:, 8:16, :], strip[:, 0:8, :])

            # ---- 3 GCN layers
            for l in range(3):
                bcol = biash_sb[:, l + 1:l + 2]
                gp = ps.tile([H, G], DT, tag="mm")
                nc.tensor.matmul(gp[:], wl_sb[:, l, :], v[:], start=True, stop=True)
                v2 = chainp.tile([H, G], DT, tag="v")
                nc.scalar.activation(v2[:], gp[:], AF.Relu, bias=bcol, scale=float(_CINT))

                hs = hsp.tile([H, S, G], DT, tag="hs")
                for c in range(8):
                    hp = ps.tile([H, 512], DT, tag="mm")
                    nc.tensor.matmul(
                        hp[:], wl_sb[:, l, :], strip[:, 2 * c:2 * c + 2, :],
                        start=True, stop=True,
                    )
                    nc.scalar.copy(hs[:, 2 * c:2 * c + 2, :], hp[:])

                # banded combine: pre[i] = sum_j Ahat[i,j] * hs[j]
                a1 = scr.tile([H, 10, G], DT, tag="a1")
                a2 = scr.tile([H, 10, G], DT, tag="a2")
                pre = scr.tile([H, 14, G], DT, tag="pre")
                nc.vector.tensor_tensor(a1[:], hs[:, 2:12, :], hs[:, 3:13, :], ALU.add)
                nc.vector.tensor_tensor(a2[:], hs[:, 4:14, :], hs[:, 5:15, :], ALU.add)
                nc.vector.tensor_tensor(a1[:], a1[:], a2[:], ALU.add)
                nc.vector.tensor_tensor(pre[:, 4:14, :], a1[:], hs[:, 6:16, :], ALU.add)
                for i in range(4):
                    js = [j for j in range(S) if abs(i - j) <= 2]
                    for t, j in enumerate(js):
                        cval = float(_BM[i, j])
                        if t == 0:
                            nc.vector.tensor_scalar(
                                pre[:, i, :], hs[:, j, :], cval, None, ALU.mult
                            )
                        else:
                            nc.vector.scalar_tensor_tensor(
                                pre[:, i, :], hs[:, j, :], cval, pre[:, i, :],
                                ALU.mult, ALU.add,
                            )
                strip2 = stripp.tile([H, S, G], DT, tag="strip")
                nc.scalar.activation(strip2[:, 0:4, :], pre[:, 0:4, :], AF.Relu, bias=bcol, scale=1.0)
                nc.scalar.activation(strip2[:, 4:14, :], pre[:, 4:14, :], AF.Relu, bias=bcol, scale=float(_Q))
                nc.gpsimd.tensor_copy(strip2[:, 14, :], v2[:])
                nc.gpsimd.tensor_copy(strip2[:, 15, :], v2[:])
                v, strip = v2, strip2

            # ---- interior output row per graph: oi[o, g] = W_out.T @ v + b_out
            op_ = ps.tile([O, G], DT, tag="mm")
            nc.tensor.matmul(op_[:], wout_sb[:], v[:], start=True, stop=True)
            oi = outp.tile([O, G], DT)
            nc.scalar.activation(oi[:], op_[:], AF.Identity, bias=bout_sb[:], scale=1.0)
            intg = outp.tile([128, 2, O], DT)
            for gc in range(2):
                tp = ps.tile([128, 128], DT, tag="tr")
                nc.tensor.transpose(tp[:], oi[:, gc * 128:(gc + 1) * 128], ident_sb[:])
                nc.vector.tensor_copy(intg[:, gc, :], tp[:])

            # ---- interior write: out[g, 8:248, :] = broadcast of intg row
            if USE_BCAST:
                for gc in range(2):
                    dst = out.ap()[gc * 128:(gc + 1) * 128, 8:248, :]
                    src = intg[:, gc:gc + 1, :]
                    srcb, _ = broadcast_tensor_aps(src, dst)
                    nc.sync.dma_start(out=dst, in_=srcb)
            else:
                R = 40
                rep = outp.tile([128, 2, R * O], DT)
                for gc in range(2):
                    nc.gpsimd.tensor_copy(rep[:, gc, 0:O], intg[:, gc, :])
                    w = O
                    while w < R * O:
                        cw = min(w, R * O - w)
                        nc.gpsimd.tensor_copy(rep[:, gc, w:w + cw], rep[:, gc, 0:cw])
                        w += cw
                    for k in range(240 // R):
                        dst = out.ap()[gc * 128:(gc + 1) * 128, 8 + k * R:8 + (k + 1) * R, :]
                        nc.sync.dma_start(out=dst, in_=rep[:, gc, :])

            # ---- edge rows: strip nodes 0..7 and mirrored 248..255
            edge2 = outp.tile([O, S, G], DT)
            for c in range(4):
                ep = ps.tile([O, 512], DT, tag="mm")
                nc.tensor.matmul(
                    ep[:], wout_sb[:], strip[:, 2 * c:2 * c + 2, :], start=True, stop=True
                )
                nc.scalar.activation(
                    edge2[:, 2 * c:2 * c + 2, :], ep[:], AF.Identity, bias=bout_sb[:], scale=1.0
                )
            for t in range(8):
                nc.gpsimd.tensor_copy(edge2[:, 8 + t, :], edge2[:, 7 - t, :])
            for i in range(S):
                n_i = i if i < 8 else 240 + i
                for gc in range(2):
                    tp = ps.tile([128, 128], DT, tag="tr")
                    nc.tensor.transpose(
                        tp[:], edge2[:, i, gc * 128:(gc + 1) * 128], ident_sb[:]
                    )
                    et = etp.tile([128, 128], DT, tag="et")
                    nc.vector.tensor_copy(et[:], tp[:])
                    nc.scalar.dma_start(out.ap()[gc * 128:(gc + 1) * 128, n_i, :], et[:])

    nc.compile()
    return nc


def _get_nc():
    global _NC
    if _NC is None:
        _NC = _build()
    return _NC


def _prepare_in_maps(inputs):
    f32 = lambda x: np.ascontiguousarray(np.asarray(x, dtype=np.float32))
    emb = f32(inputs["embedding"])
    w_emb = f32(inputs["W_emb"])
    b_emb = f32(inputs["b_emb"])
    conv_w = f32(inputs["conv_W"])
    conv_b = f32(inputs["conv_b"])
    w_out = f32(inputs["W_out"])
    b_out = f32(inputs["b_out"])
    bias_h = np.ascontiguousarray(
        np.stack([b_emb, conv_b[0], conv_b[1], conv_b[2]], axis=1)
    )
    b_out_t = np.ascontiguousarray(b_out[:, None])
    ident = np.eye(128, dtype=np.float32)
    shared = {
        "w_emb": w_emb,
        "conv_w": conv_w,
        "w_out": w_out,
        "bias_h": bias_h,
        "b_out_t": b_out_t,
        "ident": ident,
    }
    return [dict(shared, emb=emb[c * G:(c + 1) * G]) for c in range(CORES)]


def kernel(**inputs):
    nc = _get_nc()
    in_maps = _prepare_in_maps(inputs)
    res = run_bass_kernel_spmd(nc, in_maps, core_ids=list(range(CORES)))
    return np.concatenate([r["out"] for r in res.results], axis=0)


# revision 3
# speedup vs baseline: 1.4851x; 1.4851x over previous
"""Trainium2 Bass kernel for nn_ASTDecoder (banded-adjacency GCN stack).

Structure exploited (graph is fixed: nodes i<->i+1, i<->i+2 within each
256-node graph, plus self loops; initial node features are identical for
all nodes of a graph):
  * deg = [3,4,5,...,5,4,3]; interior rows of the normalized adjacency
    sum to exactly 1, so nodes far from the graph boundary keep a single
    per-graph value through all 3 conv layers.  After 3 layers only nodes
    0..7 and 248..255 differ from the per-graph constant; the two ends
    are mirror images of each other.
  * Per core (256 graphs): compute the per-graph constant chain and a
    narrow boundary strip exactly (strip rows that would equal the
    constant are taken from the chain, so layer l only computes rows
    0..2l+1), then write the output: 240 interior rows per graph are a
    replicated 512B row (the bulk of the 32MB per-core output traffic),
    16 edge rows come from the strip.
"""

import os
import sys

import numpy as np

for _p in ("/opt/trn_rl_repo", "/root/.axon_site/_ro/trn_rl_repo"):
    if os.path.isdir(_p) and _p not in sys.path:
        sys.path.insert(0, _p)

from concourse import bacc, mybir, tile
from concourse.bass import broadcast_tensor_aps
from concourse.bass_utils import run_bass_kernel_spmd

B, E, H, O, N = 2048, 256, 64, 128, 256
CORES = 8
G = B // CORES  # graphs per core
S = 16          # boundary strip width (nodes that can differ, per side)
R = 48          # interior replication factor (240 = 5 * 48)
AF = mybir.ActivationFunctionType
ALU = mybir.AluOpType
DT = mybir.dt.float32


def _consts():
    deg = np.full(N, 5.0, np.float32)
    deg[[0, -1]] = 3.0
    deg[[1, -2]] = 4.0
    dinv = (1.0 / np.sqrt(deg)).astype(np.float32)
    q = np.float32(dinv[128] * dinv[128])
    c_int = np.float32(np.float32(5.0) * q)
    Bm = np.zeros((S, S), np.float32)
    for i in range(S):
        for j in range(S):
            if abs(i - j) <= 2:
                Bm[i, j] = np.float32(dinv[i] * dinv[j])
    # diagonal coefficient table: coefd[di, i] = Bm[i, i + di - 2]
    coefd = np.zeros((5, S), np.float32)
    for di in range(5):
        d = di - 2
        for i in range(S):
            j = i + d
            if 0 <= j < S:
                coefd[di, i] = Bm[i, j]
    # layer-0 row sums (strip rows 0..3): output = relu(rowsum * g0 + b)
    rsum = np.array([Bm[i, :].sum(dtype=np.float64) for i in range(4)], np.float32)
    return q, c_int, Bm, coefd, rsum


_Q, _CINT, _BM, _COEFD, _RSUM = _consts()
_NC = None


def _build():
    nc = bacc.Bacc("TRN2", target_bir_lowering=False, debug=False, num_devices=CORES)
    emb = nc.dram_tensor("emb", [G, E], DT, kind="ExternalInput")
    w_emb = nc.dram_tensor("w_emb", [E, H], DT, kind="ExternalInput")
    conv_w = nc.dram_tensor("conv_w", [3, H, H], DT, kind="ExternalInput")
    w_out = nc.dram_tensor("w_out", [H, O], DT, kind="ExternalInput")
    bias_h = nc.dram_tensor("bias_h", [H, 4], DT, kind="ExternalInput")
    b_out_t = nc.dram_tensor("b_out_t", [O, 1], DT, kind="ExternalInput")
    ident = nc.dram_tensor("ident", [128, 128], DT, kind="ExternalInput")
    coefd = nc.dram_tensor("coefd", [H, 5, S, 1], DT, kind="ExternalInput")
    out = nc.dram_tensor("out", [G, N, O], DT, kind="ExternalOutput")

    with tile.TileContext(nc) as tc:
        with (
            tc.tile_pool(name="const", bufs=1) as constp,
            tc.tile_pool(name="embp", bufs=1) as embp,
            tc.tile_pool(name="stripp", bufs=1) as stripp,
            tc.tile_pool(name="scr", bufs=1) as scr,
            tc.tile_pool(name="chain", bufs=2) as chainp,
            tc.tile_pool(name="outp", bufs=1) as outp,
            tc.tile_pool(name="ps", bufs=2, space="PSUM") as ps,
        ):
            # ---- constants into SBUF
            wemb_sb = constp.tile([128, 2, H], DT)
            nc.sync.dma_start(wemb_sb[:], w_emb.ap().rearrange("(c p) h -> p c h", p=128))
            wl_sb = constp.tile([H, 3, H], DT)
            nc.sync.dma_start(wl_sb[:], conv_w.ap().rearrange("l h k -> h l k"))
            wout_sb = constp.tile([H, O], DT)
            nc.sync.dma_start(wout_sb[:], w_out.ap())
            biash_sb = constp.tile([H, 4], DT)
            nc.sync.dma_start(biash_sb[:], bias_h.ap())
            bout_sb = constp.tile([O, 1], DT)
            nc.sync.dma_start(bout_sb[:], b_out_t.ap())
            ident_sb = constp.tile([128, 128], DT)
            nc.sync.dma_start(ident_sb[:], ident.ap())
            coefd_sb = constp.tile([H, 5, S, 1], DT)
            nc.sync.dma_start(coefd_sb[:], coefd.ap())

            # ---- embedding load + PE transpose to [e, g]
            emb_sb = embp.tile([128, 2, E], DT)
            nc.sync.dma_start(emb_sb[:], emb.ap().rearrange("(c p) e -> p c e", p=128))
            embT = embp.tile([128, 2, G], DT)
            for ec in range(2):
                for gc in range(2):
                    tp = ps.tile([128, 128], DT, tag="tr")
                    nc.tensor.transpose(
                        tp[:], emb_sb[:, gc, ec * 128:(ec + 1) * 128], ident_sb[:]
                    )
                    nc.vector.tensor_copy(embT[:, ec, gc * 128:(gc + 1) * 128], tp[:])

            # ---- init = W_emb.T @ emb.T + b_emb   (layout [h, g])
            ip = ps.tile([H, G], DT, tag="mm")
            nc.tensor.matmul(ip[:], wemb_sb[:, 0, :], embT[:, 0, :], start=True, stop=False)
            nc.tensor.matmul(ip[:], wemb_sb[:, 1, :], embT[:, 1, :], start=False, stop=True)
            v = chainp.tile([H, G], DT, tag="v")
            nc.scalar.activation(v[:], ip[:], AF.Identity, bias=biash_sb[:, 0:1], scale=1.0)

            def diag_combine(pre, hs, nrows):
                """pre[:, i, :] = sum_d coefd[d, i] * hs[:, i+d-2, :], rows 0..nrows."""
                tmp = scr.tile([H, 8, G], DT, tag="tmp")
                for di in (2, 0, 1, 3, 4):  # center diagonal first: full-range init
                    d = di - 2
                    lo = max(0, -d)
                    cnt = nrows - lo
                    w_in = hs[:, lo + d:lo + d + cnt, :]
                    cf = coefd_sb[:, di, lo:lo + cnt, :]
                    cfb, _ = broadcast_tensor_aps(cf, w_in)
                    if di == 2:
                        nc.vector.tensor_tensor(pre[:, lo:nrows, :], w_in, cfb, ALU.mult)
                    else:
                        nc.vector.tensor_tensor(tmp[:, lo:nrows, :], w_in, cfb, ALU.mult)
                        nc.vector.tensor_tensor(
                            pre[:, lo:nrows, :], pre[:, lo:nrows, :],
                            tmp[:, lo:nrows, :], ALU.add,
                        )

            # ---- layer 0: strip rows 0..3 are just scaled chain values
            b0 = biash_sb[:, 1:2]
            gp0 = ps.tile([H, G], DT, tag="mm")
            nc.tensor.matmul(gp0[:], wl_sb[:, 0, :], v[:], start=True, stop=True)
            v1 = chainp.tile([H, G], DT, tag="v")
            nc.scalar.activation(v1[:], gp0[:], AF.Relu, bias=b0, scale=float(_CINT))
            strip1 = stripp.tile([H, 4, G], DT, tag="s1")
            for i in range(4):
                nc.scalar.activation(
                    strip1[:, i, :], gp0[:], AF.Relu, bias=b0, scale=float(_RSUM[i])
                )

            # ---- layer 1: hs rows 0..3 real, rows 4..7 = g1; combine rows 0..5
            b1 = biash_sb[:, 2:3]
            gp1 = ps.tile([H, G], DT, tag="mm")
            nc.tensor.matmul(gp1[:], wl_sb[:, 1, :], v1[:], start=True, stop=True)
            v2 = chainp.tile([H, G], DT, tag="v")
            nc.scalar.activation(v2[:], gp1[:], AF.Relu, bias=b1, scale=float(_CINT))
            hs1 = stripp.tile([H, 8, G], DT, tag="hs1")
            for c in range(2):
                hp = ps.tile([H, 512], DT, tag="mm")
                nc.tensor.matmul(
                    hp[:], wl_sb[:, 1, :], strip1[:, 2 * c:2 * c + 2, :],
                    start=True, stop=True,
                )
                nc.scalar.copy(hs1[:, 2 * c:2 * c + 2, :], hp[:])
            nc.scalar.copy(hs1[:, 4, :], gp1[:])
            nc.vector.tensor_copy(hs1[:, 5, :], hs1[:, 4, :])
            nc.vector.tensor_copy(hs1[:, 6:8, :], hs1[:, 4:6, :])
            pre1 = scr.tile([H, 8, G], DT, tag="pre")
            diag_combine(pre1, hs1, 6)
            strip2 = stripp.tile([H, 6, G], DT, tag="s2")
            nc.scalar.activation(strip2[:], pre1[:, 0:6, :], AF.Relu, bias=b1, scale=1.0)

            # ---- layer 2: hs rows 0..5 real, rows 6..9 = g2; combine rows 0..7
            b2 = biash_sb[:, 3:4]
            gp2 = ps.tile([H, G], DT, tag="mm")
            nc.tensor.matmul(gp2[:], wl_sb[:, 2, :], v2[:], start=True, stop=True)
            v3 = chainp.tile([H, G], DT, tag="v")
            nc.scalar.activation(v3[:], gp2[:], AF.Relu, bias=b2, scale=float(_CINT))
            hs2 = stripp.tile([H, 10, G], DT, tag="hs2")
            for c in range(3):
                hp = ps.tile([H, 512], DT, tag="mm")
                nc.tensor.matmul(
                    hp[:], wl_sb[:, 2, :], strip2[:, 2 * c:2 * c + 2, :],
                    start=True, stop=True,
                )
                nc.scalar.copy(hs2[:, 2 * c:2 * c + 2, :], hp[:])
            nc.scalar.copy(hs2[:, 6, :], gp2[:])
            nc.vector.tensor_copy(hs2[:, 7, :], hs2[:, 6, :])
            nc.vector.tensor_copy(hs2[:, 8:10, :], hs2[:, 6:8, :])
            pre2 = scr.tile([H, 8, G], DT, tag="pre")
            diag_combine(pre2, hs2, 8)
            strip3 = stripp.tile([H, 8, G], DT, tag="s3")
            nc.scalar.activation(strip3[:], pre2[:], AF.Relu, bias=b2, scale=1.0)

            # ---- interior output row per graph: oi[o, g] = W_out.T @ v3 + b_out
            op_ = ps.tile([O, G], DT, tag="mm")
            nc.tensor.matmul(op_[:], wout_sb[:], v3[:], start=True, stop=True)
            oi = outp.tile([O, G], DT)
            nc.scalar.activation(oi[:], op_[:], AF.Identity, bias=bout_sb[:], scale=1.0)
            intg = outp.tile([128, 2, O], DT)
            for gc in range(2):
                tp = ps.tile([128, 128], DT, tag="tr")
                nc.tensor.transpose(tp[:], oi[:, gc * 128:(gc + 1) * 128], ident_sb[:])
                nc.vector.tensor_copy(intg[:, gc, :], tp[:])

            # replicate each graph's interior row R times, then 5 big DMAs/chunk
            rep = outp.tile([128, 2, R * O], DT)
            for gc in range(2):
                nc.vector.tensor_copy(rep[:, gc, 0:O], intg[:, gc, :])
                w = O
                while w < R * O:
                    cw = min(w, R * O - w)
                    nc.vector.tensor_copy(rep[:, gc, w:w + cw], rep[:, gc, 0:cw])
                    w += cw
                for k in range(240 // R):
                    dst = out.ap()[gc * 128:(gc + 1) * 128, 8 + k * R:8 + (k + 1) * R, :]
                    nc.sync.dma_start(out=dst, in_=rep[:, gc, :])

            # ---- edge rows: strip nodes 0..7 and mirrored 248..255
            edge2 = outp.tile([O, 8, G], DT)
            for c in range(4):
                ep = ps.tile([O, 512], DT, tag="mm")
                nc.tensor.matmul(
                    ep[:], wout_sb[:], strip3[:, 2 * c:2 * c + 2, :], start=True, stop=True
                )
                nc.scalar.activation(
                    edge2[:, 2 * c:2 * c + 2, :], ep[:], AF.Identity, bias=bout_sb[:], scale=1.0
                )
            for gc in range(2):
                gs = slice(gc * 128, (gc + 1) * 128)
                eL = outp.tile([128, 8, O], DT, tag=f"eL{gc}")
                eR = outp.tile([128, 8, O], DT, tag=f"eR{gc}")
                for i in range(8):
                    tp = ps.tile([128, 128], DT, tag="tr")
                    nc.tensor.transpose(tp[:], edge2[:, i, gs], ident_sb[:])
                    nc.vector.tensor_copy(eL[:, i, :], tp[:])
                    tp2 = ps.tile([128, 128], DT, tag="tr")
                    nc.tensor.transpose(tp2[:], edge2[:, 7 - i, gs], ident_sb[:])
                    nc.vector.tensor_copy(eR[:, i, :], tp2[:])
                nc.scalar.dma_start(out.ap()[gs, 0:8, :], eL[:])
                nc.scalar.dma_start(out.ap()[gs, 248:256, :], eR[:])

    nc.compile()
    return nc


def _get_nc():
    global _NC
    if _NC is None:
        _NC = _build()
    return _NC


def _prepare_in_maps(inputs):
    f32 = lambda x: np.ascontiguousarray(np.asarray(x, dtype=np.float32))
    emb = f32(inputs["embedding"])
    w_emb = f32(inputs["W_emb"])
    b_emb = f32(inputs["b_emb"])
    conv_w = f32(inputs["conv_W"])
    conv_b = f32(inputs["conv_b"])
    w_out = f32(inputs["W_out"])
    b_out = f32(inputs["b_out"])
    bias_h = np.ascontiguousarray(
        np.stack([b_emb, conv_b[0], conv_b[1], conv_b[2]], axis=1)
    )
    b_out_t = np.ascontiguousarray(b_out[:, None])
    ident = np.eye(128, dtype=np.float32)
    coefd = np.ascontiguousarray(
        np.broadcast_to(_COEFD[None, :, :, None], (H, 5, S, 1)).astype(np.float32)
    )
    shared = {
        "w_emb": w_emb,
        "conv_w": conv_w,
        "w_out": w_out,
        "bias_h": bias_h,
        "b_out_t": b_out_t,
        "ident": ident,
        "coefd": coefd,
    }
    return [dict(shared, emb=emb[c * G:(c + 1) * G]) for c in range(CORES)]


def kernel(**inputs):
    nc = _get_nc()
    in_maps = _prepare_in_maps(inputs)
    res = run_bass_kernel_spmd(nc, in_maps, core_ids=list(range(CORES)))
    return np.concatenate([r["out"] for r in res.results], axis=0)


# revision 4
# speedup vs baseline: 1.9809x; 1.3338x over previous
"""Trainium2 Bass kernel for nn_ASTDecoder (banded-adjacency GCN stack).

Structure exploited (graph is fixed: nodes i<->i+1, i<->i+2 within each
256-node graph, plus self loops; initial node features are identical for
all nodes of a graph):
  * deg = [3,4,5,...,5,4,3]; interior rows of the normalized adjacency
    sum to exactly 1, so nodes far from the graph boundary keep a single
    per-graph value through all 3 conv layers.  After 3 layers only nodes
    0..7 and 248..255 differ from the per-graph constant; the two ends
    are mirror images of each other.
  * Per core (256 graphs): compute the per-graph constant chain and a
    narrow boundary strip exactly (strip rows that would equal the
    constant are taken from the chain, so layer l only computes rows
    0..2l+1), then write the output: 240 interior rows per graph are a
    replicated 512B row (the bulk of the 32MB per-core output traffic),
    16 edge rows come from the strip.

Program order is chosen so the interior-output path (chain -> W_out ->
transpose -> replicate -> 30MB of DMA) is emitted before any strip work:
engine instruction order is fixed at schedule time, and the big DMAs
must start as early as possible; the strip/edge work overlaps them.
"""

import os
import sys

import numpy as np

for _p in ("/opt/trn_rl_repo", "/root/.axon_site/_ro/trn_rl_repo"):
    if os.path.isdir(_p) and _p not in sys.path:
        sys.path.insert(0, _p)

from concourse import bacc, mybir, tile
from concourse.bass import broadcast_tensor_aps
from concourse.bass_utils import run_bass_kernel_spmd

B, E, H, O, N = 2048, 256, 64, 128, 256
CORES = 8
G = B // CORES  # graphs per core
S = 16          # boundary strip width (nodes that can differ, per side)
R = 48          # interior replication factor (240 = 5 * 48)
AF = mybir.ActivationFunctionType
ALU = mybir.AluOpType
DT = mybir.dt.float32


def _consts():
    deg = np.full(N, 5.0, np.float32)
    deg[[0, -1]] = 3.0
    deg[[1, -2]] = 4.0
    dinv = (1.0 / np.sqrt(deg)).astype(np.float32)
    q = np.float32(dinv[128] * dinv[128])
    c_int = np.float32(np.float32(5.0) * q)
    Bm = np.zeros((S, S), np.float32)
    for i in range(S):
        for j in range(S):
            if abs(i - j) <= 2:
                Bm[i, j] = np.float32(dinv[i] * dinv[j])
    # diagonal coefficient table: coefd[di, i] = Bm[i, i + di - 2]
    coefd = np.zeros((5, S), np.float32)
    for di in range(5):
        d = di - 2
        for i in range(S):
            j = i + d
            if 0 <= j < S:
                coefd[di, i] = Bm[i, j]
    # layer-0 row sums (strip rows 0..3): output = relu(rowsum * g0 + b)
    rsum = np.array([Bm[i, :].sum(dtype=np.float64) for i in range(4)], np.float32)
    return q, c_int, Bm, coefd, rsum


_Q, _CINT, _BM, _COEFD, _RSUM = _consts()
_NC = None


def _build():
    nc = bacc.Bacc("TRN2", target_bir_lowering=False, debug=False, num_devices=CORES)
    emb = nc.dram_tensor("emb", [G, E], DT, kind="ExternalInput")
    w_emb = nc.dram_tensor("w_emb", [E, H], DT, kind="ExternalInput")
    conv_w = nc.dram_tensor("conv_w", [3, H, H], DT, kind="ExternalInput")
    w_out = nc.dram_tensor("w_out", [H, O], DT, kind="ExternalInput")
    bias_h = nc.dram_tensor("bias_h", [H, 4], DT, kind="ExternalInput")
    b_out_t = nc.dram_tensor("b_out_t", [O, 1], DT, kind="ExternalInput")
    ident = nc.dram_tensor("ident", [128, 128], DT, kind="ExternalInput")
    coefd = nc.dram_tensor("coefd", [H, 5, S, 1], DT, kind="ExternalInput")
    out = nc.dram_tensor("out", [G, N, O], DT, kind="ExternalOutput")

    with tile.TileContext(nc) as tc:
        with (
            tc.tile_pool(name="const", bufs=1) as constp,
            tc.tile_pool(name="embp", bufs=1) as embp,
            tc.tile_pool(name="stripp", bufs=1) as stripp,
            tc.tile_pool(name="scr", bufs=1) as scr,
            tc.tile_pool(name="chain", bufs=1) as chainp,
            tc.tile_pool(name="outp", bufs=1) as outp,
            tc.tile_pool(name="ps", bufs=2, space="PSUM") as ps,
        ):
            # ---- constants into SBUF
            wemb_sb = constp.tile([128, 2, H], DT)
            nc.sync.dma_start(wemb_sb[:], w_emb.ap().rearrange("(c p) h -> p c h", p=128))
            wl_sb = constp.tile([H, 3, H], DT)
            nc.sync.dma_start(wl_sb[:], conv_w.ap().rearrange("l h k -> h l k"))
            wout_sb = constp.tile([H, O], DT)
            nc.sync.dma_start(wout_sb[:], w_out.ap())
            biash_sb = constp.tile([H, 4], DT)
            nc.sync.dma_start(biash_sb[:], bias_h.ap())
            bout_sb = constp.tile([O, 1], DT)
            nc.sync.dma_start(bout_sb[:], b_out_t.ap())
            ident_sb = constp.tile([128, 128], DT)
            nc.sync.dma_start(ident_sb[:], ident.ap())
            coefd_sb = constp.tile([H, 5, S, 1], DT)
            nc.sync.dma_start(coefd_sb[:], coefd.ap())

            # ---- embedding load + PE transpose to [e, g]
            emb_sb = embp.tile([128, 2, E], DT)
            nc.sync.dma_start(emb_sb[:], emb.ap().rearrange("(c p) e -> p c e", p=128))
            embT = embp.tile([128, 2, G], DT)
            for ec in range(2):
                for gc in range(2):
                    tp = ps.tile([128, 128], DT, tag="tr")
                    nc.tensor.transpose(
                        tp[:], emb_sb[:, gc, ec * 128:(ec + 1) * 128], ident_sb[:]
                    )
                    nc.vector.tensor_copy(embT[:, ec, gc * 128:(gc + 1) * 128], tp[:])

            # ---- init = W_emb.T @ emb.T + b_emb   (layout [h, g])
            ip = ps.tile([H, G], DT, tag="mm")
            nc.tensor.matmul(ip[:], wemb_sb[:, 0, :], embT[:, 0, :], start=True, stop=False)
            nc.tensor.matmul(ip[:], wemb_sb[:, 1, :], embT[:, 1, :], start=False, stop=True)
            v = chainp.tile([H, G], DT, tag="v0")
            nc.scalar.activation(v[:], ip[:], AF.Identity, bias=biash_sb[:, 0:1], scale=1.0)

            # ---- per-graph constant chain (gp* psums kept alive for the strip)
            b0, b1, b2 = (biash_sb[:, k:k + 1] for k in (1, 2, 3))
            gp0 = ps.tile([H, G], DT, tag="g0", bufs=1)
            nc.tensor.matmul(gp0[:], wl_sb[:, 0, :], v[:], start=True, stop=True)
            v1 = chainp.tile([H, G], DT, tag="v1")
            nc.scalar.activation(v1[:], gp0[:], AF.Relu, bias=b0, scale=float(_CINT))
            gp1 = ps.tile([H, G], DT, tag="g1", bufs=1)
            nc.tensor.matmul(gp1[:], wl_sb[:, 1, :], v1[:], start=True, stop=True)
            v2 = chainp.tile([H, G], DT, tag="v2")
            nc.scalar.activation(v2[:], gp1[:], AF.Relu, bias=b1, scale=float(_CINT))
            gp2 = ps.tile([H, G], DT, tag="g2", bufs=1)
            nc.tensor.matmul(gp2[:], wl_sb[:, 2, :], v2[:], start=True, stop=True)
            v3 = chainp.tile([H, G], DT, tag="v3")
            nc.scalar.activation(v3[:], gp2[:], AF.Relu, bias=b2, scale=float(_CINT))

            # ---- interior output row per graph: oi[o, g] = W_out.T @ v3 + b_out
            op_ = ps.tile([O, G], DT, tag="mm")
            nc.tensor.matmul(op_[:], wout_sb[:], v3[:], start=True, stop=True)
            oi = outp.tile([O, G], DT)
            nc.scalar.activation(oi[:], op_[:], AF.Identity, bias=bout_sb[:], scale=1.0)
            intg = outp.tile([128, 2, O], DT)
            rep = outp.tile([128, 2, R * O], DT)
            for gc in range(2):
                tp = ps.tile([128, 128], DT, tag="tr")
                nc.tensor.transpose(tp[:], oi[:, gc * 128:(gc + 1) * 128], ident_sb[:])
                nc.vector.tensor_copy(intg[:, gc, :], tp[:])
                # replicate each graph's interior row R times -> 5 big DMAs
                nc.vector.tensor_copy(rep[:, gc, 0:O], intg[:, gc, :])
                w = O
                while w < R * O:
                    cw = min(w, R * O - w)
                    nc.vector.tensor_copy(rep[:, gc, w:w + cw], rep[:, gc, 0:cw])
                    w += cw
                for k in range(240 // R):
                    dst = out.ap()[gc * 128:(gc + 1) * 128, 8 + k * R:8 + (k + 1) * R, :]
                    nc.sync.dma_start(out=dst, in_=rep[:, gc, :])

            # ---- boundary strip (overlaps the interior DMAs)
            def diag_combine(pre, hs, nrows):
                """pre[:, i, :] = sum_d coefd[d, i] * hs[:, i+d-2, :], rows 0..nrows."""
                tmp = scr.tile([H, 8, G], DT, tag="tmp")
                for di in (2, 0, 1, 3, 4):  # center diagonal first: full-range init
                    d = di - 2
                    lo = max(0, -d)
                    cnt = nrows - lo
                    w_in = hs[:, lo + d:lo + d + cnt, :]
                    cf = coefd_sb[:, di, lo:lo + cnt, :]
                    cfb, _ = broadcast_tensor_aps(cf, w_in)
                    if di == 2:
                        nc.vector.tensor_tensor(pre[:, lo:nrows, :], w_in, cfb, ALU.mult)
                    else:
                        nc.vector.tensor_tensor(tmp[:, lo:nrows, :], w_in, cfb, ALU.mult)
                        nc.vector.tensor_tensor(
                            pre[:, lo:nrows, :], pre[:, lo:nrows, :],
                            tmp[:, lo:nrows, :], ALU.add,
                        )

            # layer 0: strip rows 0..3 are just scaled chain values
            strip1 = stripp.tile([H, 4, G], DT, tag="s1")
            for i in range(4):
                nc.scalar.activation(
                    strip1[:, i, :], gp0[:], AF.Relu, bias=b0, scale=float(_RSUM[i])
                )

            # layer 1: hs rows 0..3 real, rows 4..7 = g1; combine rows 0..5
            hs1 = stripp.tile([H, 8, G], DT, tag="hs1")
            for c in range(2):
                hp = ps.tile([H, 512], DT, tag="mm")
                nc.tensor.matmul(
                    hp[:], wl_sb[:, 1, :], strip1[:, 2 * c:2 * c + 2, :],
                    start=True, stop=True,
                )
                nc.scalar.copy(hs1[:, 2 * c:2 * c + 2, :], hp[:])
            nc.scalar.copy(hs1[:, 4, :], gp1[:])
            nc.vector.tensor_copy(hs1[:, 5, :], hs1[:, 4, :])
            nc.vector.tensor_copy(hs1[:, 6:8, :], hs1[:, 4:6, :])
            pre1 = scr.tile([H, 8, G], DT, tag="pre")
            diag_combine(pre1, hs1, 6)
            strip2 = stripp.tile([H, 6, G], DT, tag="s2")
            nc.scalar.activation(strip2[:], pre1[:, 0:6, :], AF.Relu, bias=b1, scale=1.0)

            # layer 2: hs rows 0..5 real, rows 6..9 = g2; combine rows 0..7
            hs2 = stripp.tile([H, 10, G], DT, tag="hs2")
            for c in range(3):
                hp = ps.tile([H, 512], DT, tag="mm")
                nc.tensor.matmul(
                    hp[:], wl_sb[:, 2, :], strip2[:, 2 * c:2 * c + 2, :],
                    start=True, stop=True,
                )
                nc.scalar.copy(hs2[:, 2 * c:2 * c + 2, :], hp[:])
            nc.scalar.copy(hs2[:, 6, :], gp2[:])
            nc.vector.tensor_copy(hs2[:, 7, :], hs2[:, 6, :])
            nc.vector.tensor_copy(hs2[:, 8:10, :], hs2[:, 6:8, :])
            pre2 = scr.tile([H, 8, G], DT, tag="pre")
            diag_combine(pre2, hs2, 8)
            strip3 = stripp.tile([H, 8, G], DT, tag="s3")
            nc.scalar.activation(strip3[:], pre2[:], AF.Relu, bias=b2, scale=1.0)

            # ---- edge rows: strip nodes 0..7 and mirrored 248..255
            edge2 = outp.tile([O, 8, G], DT)
            for c in range(4):
                ep = ps.tile([O, 512], DT, tag="mm")
                nc.tensor.matmul(
                    ep[:], wout_sb[:], strip3[:, 2 * c:2 * c + 2, :], start=True, stop=True
                )
                nc.scalar.activation(
                    edge2[:, 2 * c:2 * c + 2, :], ep[:], AF.Identity, bias=bout_sb[:], scale=1.0
                )
            for gc in range(2):
                gs = slice(gc * 128, (gc + 1) * 128)
                eL = outp.tile([128, 8, O], DT, tag=f"eL{gc}")
                eR = outp.tile([128, 8, O], DT, tag=f"eR{gc}")
                for i in range(8):
                    tp = ps.tile([128, 128], DT, tag="tr")
                    nc.tensor.transpose(tp[:], edge2[:, i, gs], ident_sb[:])
                    nc.vector.tensor_copy(eL[:, i, :], tp[:])
                    tp2 = ps.tile([128, 128], DT, tag="tr")
                    nc.tensor.transpose(tp2[:], edge2[:, 7 - i, gs], ident_sb[:])
                    nc.vector.tensor_copy(eR[:, i, :], tp2[:])
                nc.scalar.dma_start(out.ap()[gs, 0:8, :], eL[:])
                nc.scalar.dma_start(out.ap()[gs, 248:256, :], eR[:])

    nc.compile()
    return nc


def _get_nc():
    global _NC
    if _NC is None:
        _NC = _build()
    return _NC


def _prepare_in_maps(inputs):
    f32 = lambda x: np.ascontiguousarray(np.asarray(x, dtype=np.float32))
    emb = f32(inputs["embedding"])
    w_emb = f32(inputs["W_emb"])
    b_emb = f32(inputs["b_emb"])
    conv_w = f32(inputs["conv_W"])
    conv_b = f32(inputs["conv_b"])
    w_out = f32(inputs["W_out"])
    b_out = f32(inputs["b_out"])
    bias_h = np.ascontiguousarray(
        np.stack([b_emb, conv_b[0], conv_b[1], conv_b[2]], axis=1)
    )
    b_out_t = np.ascontiguousarray(b_out[:, None])
    ident = np.eye(128, dtype=np.float32)
    coefd = np.ascontiguousarray(
        np.broadcast_to(_COEFD[None, :, :, None], (H, 5, S, 1)).astype(np.float32)
    )
    shared = {
        "w_emb": w_emb,
        "conv_w": conv_w,
        "w_out": w_out,
        "bias_h": bias_h,
        "b_out_t": b_out_t,
        "ident": ident,
        "coefd": coefd,
    }
    return [dict(shared, emb=emb[c * G:(c + 1) * G]) for c in range(CORES)]


def kernel(**inputs):
    nc = _get_nc()
    in_maps = _prepare_in_maps(inputs)
    res = run_bass_kernel_spmd(nc, in_maps, core_ids=list(range(CORES)))
    return np.concatenate([r["out"] for r in res.results], axis=0)
